# revision 1
# baseline (speedup 1.0000x reference)
"""Dev kernel.py — imports attn_kernel; will be inlined for submission."""

import numpy as np

import attn_kernel as ak

N_CORES = 8
_CACHE = {}


def kernel(**inputs) -> np.ndarray:
    shared, xTs = ak.host_prep(inputs, probs_bf16=True)
    if "nc" not in _CACHE:
        _CACHE["nc"] = ak.build_nc(probs_bf16=True, band_margin=23.0)
    nc = _CACHE["nc"]
    in_maps = [dict(shared, xT=xTs[c]) for c in range(N_CORES)]
    from concourse.bass_utils import run_bass_kernel_spmd

    res = run_bass_kernel_spmd(nc, in_maps, core_ids=list(range(N_CORES)))
    outs = [res.results[c]["out"].reshape(ak.BPC, ak.S, ak.D) for c in range(N_CORES)]
    return np.concatenate(outs, axis=0).astype(np.float32)
